# revision 1
# baseline (speedup 1.0000x reference)
"""3x3 median blur on Trainium2, data-parallel across 8 NeuronCores.

Input:  image (16, 3, 512, 512) float32
Output: median-blur(3x3, zero-padded) same shape.

Strategy:
- Shard batch across 8 cores: core c handles images [2c, 2c+2) -> 6 channel
  planes of 512x512 each.
- Host pads each plane to 514x514 with zeros and restages it into device
  layout [pass][partition][slab]: per pass two planes, each split into 64
  row-chunks of 8 output rows; partition p holds a 10-row x 514-col slab
  (8 output rows + 1 halo row each side). Restaging makes every DMA a plain
  [128 partitions x contiguous] transfer at full SDMA rate.
- Device kernel (per core): 3 passes x 2 sub-passes; exact fp32 median-of-9
  via separable sorting network on the vector engine:
    vertical: sort each 3-tall column into (lo, mid, hi) using shared
      adjacent-row min/max pairs;
    horizontal: median9 = med3(max3(lo), med3(mid), min3(hi)) with shared
      even/odd column pairs.
  ~15 tensor_tensor min/max element-cycles per output pixel, fp32-exact;
  odd/even phases are fused into single instructions via 3-dim access
  patterns (broadcast and negative strides).
- Output staged as [pass][sub-pass][partition][4*512]; host scatters back.
"""

import hashlib
import os
import shutil
import sys

if "/opt/trn_rl_repo" not in sys.path:
    sys.path.insert(0, "/opt/trn_rl_repo")

import numpy as np

import concourse.bass as bass
import concourse.tile as tile
from concourse import bacc, mybir
from concourse.bass_utils import run_bass_kernel_spmd

F32 = mybir.dt.float32
MAX = mybir.AluOpType.max
MIN = mybir.AluOpType.min

N_CORES = 8
B, C, H, W = 16, 3, 512, 512
PLANES = (B * C) // N_CORES  # 6 planes per core
PH, PW = H + 2, W + 2  # 514, 514

N_PASSES = PLANES // 2  # 2 planes per pass
CHUNK = 8  # output rows per partition per pass
SLAB = CHUNK + 2  # input rows per partition slab
SUB = 4  # output rows per sub-pass (2 sub-passes per pass)
SLABE = SLAB * PW  # slab elems per partition (5140)
OUTE = SUB * W  # output elems per partition per sub-pass (2048)
NP = PW // 2  # 257 even/odd column pairs
NH = W // 2  # 256

_CACHED = {}

_NEFF_CACHE_DIR = "/tmp/bass_neff_cache"


def _install_neff_cache():
    """Memoise walrus compiles on disk, keyed by the BIR json hash."""
    if _CACHED.get("neff_cache"):
        return
    import concourse.bass2jax as b2j
    import concourse.bass_utils as bu

    orig = bu.compile_bir_kernel

    def cached_compile(bir_json, tmpdir, neff_name="file.neff"):
        key = hashlib.sha256(bir_json).hexdigest()
        cpath = os.path.join(_NEFF_CACHE_DIR, f"{key}.neff")
        dst = os.path.join(tmpdir, neff_name)
        if os.path.exists(cpath):
            shutil.copy(cpath, dst)
            return dst
        p = orig(bir_json, tmpdir, neff_name)
        try:
            os.makedirs(_NEFF_CACHE_DIR, exist_ok=True)
            tmp = cpath + ".tmp"
            shutil.copy(p, tmp)
            os.replace(tmp, cpath)
        except OSError:
            pass
        return p

    bu.compile_bir_kernel = cached_compile
    b2j.compile_bir_kernel = cached_compile
    _CACHED["neff_cache"] = True


def _ap(apref, off, dims):
    """View into a tile AP with explicit [step, num] free dims."""
    part = list(apref.ap[0])
    return bass.AP(apref.tensor, apref.offset + off, [part] + [list(d) for d in dims])


def _dram(handle, off, dims):
    return bass.AP(handle, off, [list(d) for d in dims])


def _build():
    nc = bacc.Bacc(
        "TRN2", target_bir_lowering=False, debug=False, num_devices=N_CORES
    )
    xin = nc.dram_tensor(
        "xs", [N_PASSES, 128, SLABE], F32, kind="ExternalInput"
    )
    yout = nc.dram_tensor(
        "ys", [N_PASSES, 2, 128, OUTE], F32, kind="ExternalOutput"
    )

    with tile.TileContext(nc) as tc:
        _body(tc, nc, xin, yout)

    nc.compile()
    return nc


def _body(tc, nc, xin, yout):
    from contextlib import ExitStack

    ctx = ExitStack()
    with ctx:
        xpool = ctx.enter_context(tc.tile_pool(name="xpool", bufs=2))
        vpool = ctx.enter_context(tc.tile_pool(name="vpool", bufs=1))
        lmh = ctx.enter_context(tc.tile_pool(name="lmh", bufs=1))
        hpool = ctx.enter_context(tc.tile_pool(name="hpool", bufs=1))
        abc = ctx.enter_context(tc.tile_pool(name="abc", bufs=1))
        fin = ctx.enter_context(tc.tile_pool(name="fin", bufs=1))
        opool = ctx.enter_context(tc.tile_pool(name="opool", bufs=2))

        tt = nc.vector.tensor_tensor

        for t in range(N_PASSES):
            X = xpool.tile([128, SLABE], F32, name="X")
            # Chunked input DMA. Chunk A covers rows 0..6 (everything the
            # first sub-pass reads) plus one element of chunk B's range; the
            # 1-element WAW overlap makes chunk B wait for chunk A, so A runs
            # at full SDMA rate and compute starts as soon as it lands.
            ca_end = 6 * PW + 1
            for e0, e1 in ((0, ca_end), (ca_end - 1, SLABE)):
                nc.sync.dma_start(
                    X[:, e0:e1],
                    _dram(
                        xin, t * 128 * SLABE + e0, [[SLABE, 128], [1, e1 - e0]]
                    ),
                )

            for sp in range(2):
                b = sp * SUB  # slab row base for this sub-pass

                def xv(r0, nrows, rstep=2):
                    return _ap(X, (b + r0) * PW, [[rstep * PW, nrows], [1, PW]])

                # ---- vertical: column sort3 -> lo, mid, hi ----
                # pairs at slab rows (b+1,b+2), (b+3,b+4); fused odd/even
                # sorts via [k=2][pol=2][514] APs: third element is
                # X[b+2k] (pol 0) or X[b+3+2k] (pol 1); output row 2k+pol.
                pmin = vpool.tile([128, 2 * PW], F32, name="pmin")
                pmax = vpool.tile([128, 2 * PW], F32, name="pmax")
                pv = [[PW, 2], [1, PW]]
                tt(_ap(pmin, 0, pv), xv(1, 2), xv(2, 2), MIN)
                tt(_ap(pmax, 0, pv), xv(1, 2), xv(2, 2), MAX)

                lo = lmh.tile([128, SUB * PW], F32, name="lo")
                mid = lmh.tile([128, SUB * PW], F32, name="mid")
                hi = lmh.tile([128, SUB * PW], F32, name="hi")
                u = vpool.tile([128, SUB * PW], F32, name="u")

                vout = [[2 * PW, 2], [PW, 2], [1, PW]]  # row 2k+pol
                vbcast = [[PW, 2], [0, 2], [1, PW]]  # pair k, pol-broadcast
                third = _ap(X, b * PW, [[2 * PW, 2], [3 * PW, 2], [1, PW]])
                pm = _ap(pmin, 0, vbcast)
                pM = _ap(pmax, 0, vbcast)
                tt(_ap(lo, 0, vout), pm, third, MIN)
                tt(_ap(hi, 0, vout), pM, third, MAX)
                tt(_ap(u, 0, vout), pM, third, MIN)
                tt(_ap(mid, 0, vout), pm, _ap(u, 0, vout), MAX)

                # ---- horizontal ----
                def cview(tl, c0, ncols, cstep=2):
                    return _ap(tl, c0, [[PW, SUB], [cstep, ncols]])

                def pview(tl, k0, nk):
                    return _ap(tl, k0, [[NP, SUB], [1, nk]])

                mlo = hpool.tile([128, SUB * NP], F32, name="mlo")
                mhi = hpool.tile([128, SUB * NP], F32, name="mhi")
                pmn = hpool.tile([128, SUB * NP], F32, name="pmn")
                pmx = hpool.tile([128, SUB * NP], F32, name="pmx")

                tt(pview(mlo, 0, NP), cview(lo, 0, NP), cview(lo, 1, NP), MAX)
                tt(pview(mhi, 0, NP), cview(hi, 0, NP), cview(hi, 1, NP), MIN)
                tt(pview(pmn, 0, NP), cview(mid, 0, NP), cview(mid, 1, NP), MIN)
                tt(pview(pmx, 0, NP), cview(mid, 0, NP), cview(mid, 1, NP), MAX)

                # fused sliding windows over output col j = pol + 2*j2:
                #   pair index k = pol + j2, third col = 2 - pol + 2*j2
                A = abc.tile([128, SUB * W], F32, name="A")
                Bt = abc.tile([128, SUB * W], F32, name="Bt")
                Ct = abc.tile([128, SUB * W], F32, name="Ct")
                u2 = hpool.tile([128, SUB * 2 * NH], F32, name="u2")

                hout = [[W, SUB], [1, 2], [2, NH]]
                hpair = lambda tl: _ap(tl, 0, [[NP, SUB], [1, 2], [1, NH]])
                hthird = lambda tl: _ap(tl, 2, [[PW, SUB], [-1, 2], [2, NH]])
                u2v = _ap(u2, 0, [[2 * NH, SUB], [NH, 2], [1, NH]])

                tt(_ap(A, 0, hout), hpair(mlo), hthird(lo), MAX)
                tt(_ap(Ct, 0, hout), hpair(mhi), hthird(hi), MIN)
                tt(u2v, hpair(pmx), hthird(mid), MIN)
                tt(_ap(Bt, 0, hout), hpair(pmn), u2v, MAX)

                # ---- final med3(A, B, C) ----
                flat = [[1, OUTE]]
                mn = fin.tile([128, OUTE], F32, name="mn")
                mx = fin.tile([128, OUTE], F32, name="mx")
                t2 = fin.tile([128, OUTE], F32, name="t2")
                res = opool.tile([128, OUTE], F32, name="res")
                tt(_ap(mn, 0, flat), _ap(A, 0, flat), _ap(Bt, 0, flat), MIN)
                tt(_ap(mx, 0, flat), _ap(A, 0, flat), _ap(Bt, 0, flat), MAX)
                tt(_ap(t2, 0, flat), _ap(mx, 0, flat), _ap(Ct, 0, flat), MIN)
                tt(_ap(res, 0, flat), _ap(mn, 0, flat), _ap(t2, 0, flat), MAX)

                nc.sync.dma_start(
                    _dram(
                        yout,
                        (t * 2 + sp) * 128 * OUTE,
                        [[OUTE, 128], [1, OUTE]],
                    ),
                    res[:, :],
                )


def _get_nc():
    if "nc" not in _CACHED:
        _install_neff_cache()
        _CACHED["nc"] = _build()
    return _CACHED["nc"]


# staged-input row gather: for each chunk c (0..63), padded rows 8c..8c+10
_ROWIDX = (np.arange(64) * CHUNK)[:, None] + np.arange(SLAB)[None, :]


def _stage_input(shard6: np.ndarray) -> np.ndarray:
    """(6, 512, 512) -> [3, 128, SLABE] staged slabs (zero-padded)."""
    padded = np.zeros((PLANES, PH, PW), dtype=np.float32)
    padded[:, 1:-1, 1:-1] = shard6
    slabs = padded[:, _ROWIDX, :]  # (6, 64, 10, 514)
    return slabs.reshape(N_PASSES, 128, SLABE)


def _unstage_output(ys: np.ndarray) -> np.ndarray:
    """[3, 2, 128, OUTE] -> (6, 512, 512)."""
    # ys[t, sp, 64h + c, r*512:...] = plane(2t+h), row 8c + 4sp + r
    arr = ys.reshape(N_PASSES, 2, 2, 64, SUB, W)  # (t, sp, h, c, r, w)
    arr = arr.transpose(0, 2, 3, 1, 4, 5)  # (t, h, c, sp, r, w)
    return arr.reshape(PLANES, H, W)


def kernel(image: np.ndarray, _trace: bool = False):
    assert image.shape == (B, C, H, W) and image.dtype == np.float32
    nc = _get_nc()

    per_core = B // N_CORES
    in_maps = []
    for c in range(N_CORES):
        shard = image[c * per_core : (c + 1) * per_core].reshape(PLANES, H, W)
        in_maps.append({"xs": _stage_input(shard)})

    res = run_bass_kernel_spmd(
        nc, in_maps, list(range(N_CORES)), trace=_trace
    )
    _CACHED["last_exec_ns"] = res.exec_time_ns

    out = np.empty((B, C, H, W), dtype=np.float32)
    for c in range(N_CORES):
        out[c * per_core : (c + 1) * per_core] = _unstage_output(
            res.results[c]["ys"]
        ).reshape(per_core, C, H, W)
    return out



# revision 2
# speedup vs baseline: 1.0005x; 1.0005x over previous
"""3x3 median blur on Trainium2, data-parallel across 8 NeuronCores (bf16).

Input:  image (16, 3, 512, 512) float32
Output: median-blur(3x3, zero-padded) same shape.

Strategy:
- Shard batch across 8 cores (2 images = 6 channel planes per core).
- Median is order-preserving under the monotone fp32->bf16 rounding, so the
  whole pipeline runs in bf16: the output equals bf16(true median), rel err
  <= 2^-9. DVE tensor_tensor then runs in the 2x packed mode (2 elem/cycle,
  measured 0.556 ns/elem) and DMA traffic halves. On this toolchain the 2x
  mode engages for ANY inner step +-1 pattern (misaligned offsets, negative
  strides, broadcast middle dims all fine); only inner step >=2 drops to 1x.
- Host stages each zero-padded 514-wide row column-DEINTERLEAVED:
  row' = [E | O], E[k]=row[2k], O[k]=row[2k+1], each half padded to 258
  (row width 516). Horizontal-window neighbours then sit at step-1 offsets:
    even out j=2k: {E[k], O[k], E[k+1]};  odd j=2k+1: {O[k], E[k+1], O[k+1]}
  so every op keeps the 2x mode (step-2 APs would halve throughput).
- Exact median-of-9 = med3(max3(lo), med3(mid), min3(hi)), ~15 min/max
  elem-ops per output with full sharing:
  vertical sort3 shares adjacent-row min/max pairs between 2 output rows;
  horizontal chains share the (O[k], E[k+1]) pair between even/odd outputs;
  even/odd output phases and the {s_lo,pmx}/{s_hi,pmn} pair ops are fused
  into single 3-free-dim-AP instructions (16 DVE ops per pass).
- Full-pass granularity: 2 planes per pass, partition p=64h+c holds a
  10-row x 516 slab (8 output rows + halo), 3 passes per core. Input DMAs
  prefetch on the Sync queue; output DMAs issue from the idle Scalar queue
  so they never block the prefetch. All min/max runs on the DVE (~99% busy;
  GPSIMD cannot execute tensor_tensor on this neuronxcc build and ScalarE
  has no two-tensor op).
"""

import hashlib
import os
import shutil
import sys

if "/opt/trn_rl_repo" not in sys.path:
    sys.path.insert(0, "/opt/trn_rl_repo")

import numpy as np
import ml_dtypes

import concourse.bass as bass
import concourse.tile as tile
from concourse import bacc, mybir
from concourse.bass_utils import run_bass_kernel_spmd

BF16 = mybir.dt.bfloat16
MAX = mybir.AluOpType.max
MIN = mybir.AluOpType.min

N_CORES = 8
B, C, H, W = 16, 3, 512, 512
PLANES = (B * C) // N_CORES  # 6 planes per core
N_PASSES = PLANES // 2  # 2 planes per pass

EW = 258  # even/odd half width (257 data + 1 pad)
RW = 2 * EW  # 516 staged row width
SLAB = 10  # input slab rows per partition (8 out + halo)
ROWS = 8  # output rows per partition per pass
SLABE = SLAB * RW  # 5160
OUTE = ROWS * RW  # 4128

_CACHED = {}

_NEFF_CACHE_DIR = "/tmp/bass_neff_cache"


def _install_neff_cache():
    """Memoise walrus compiles on disk, keyed by the BIR json hash."""
    if _CACHED.get("neff_cache"):
        return
    import concourse.bass2jax as b2j
    import concourse.bass_utils as bu

    orig = bu.compile_bir_kernel

    def cached_compile(bir_json, tmpdir, neff_name="file.neff"):
        key = hashlib.sha256(bir_json).hexdigest()
        cpath = os.path.join(_NEFF_CACHE_DIR, f"{key}.neff")
        dst = os.path.join(tmpdir, neff_name)
        if os.path.exists(cpath):
            shutil.copy(cpath, dst)
            return dst
        p = orig(bir_json, tmpdir, neff_name)
        try:
            os.makedirs(_NEFF_CACHE_DIR, exist_ok=True)
            tmp = cpath + ".tmp"
            shutil.copy(p, tmp)
            os.replace(tmp, cpath)
        except OSError:
            pass
        return p

    bu.compile_bir_kernel = cached_compile
    b2j.compile_bir_kernel = cached_compile
    _CACHED["neff_cache"] = True


def _ap(apref, off, dims):
    part = list(apref.ap[0])
    return bass.AP(apref.tensor, apref.offset + off, [part] + [list(d) for d in dims])


def _dram(handle, off, dims):
    return bass.AP(handle, off, [list(d) for d in dims])


def _build():
    nc = bacc.Bacc(
        "TRN2", target_bir_lowering=False, debug=False, num_devices=N_CORES
    )
    xin = nc.dram_tensor("xs", [N_PASSES, 128, SLABE], BF16, kind="ExternalInput")
    yout = nc.dram_tensor("ys", [N_PASSES, 128, OUTE], BF16, kind="ExternalOutput")

    with tile.TileContext(nc) as tc:
        _body(tc, nc, xin, yout)

    nc.compile()
    return nc


L = OUTE + 2  # lo/mid/hi region pitch (2 zeroed tail elems)
R = ROWS * EW  # s-tile region pitch (2064)
USE_4D = False  # 4-free-dim fused ops ({lo,u}, {Ct,u2}); needs walrus support


def _body(tc, nc, xin, yout):
    from contextlib import ExitStack

    ctx = ExitStack()
    with ctx:
        xpool = ctx.enter_context(tc.tile_pool(name="xpool", bufs=2))
        vert = ctx.enter_context(tc.tile_pool(name="vert", bufs=1))
        lmhp = ctx.enter_context(tc.tile_pool(name="lmhp", bufs=1))
        hp = ctx.enter_context(tc.tile_pool(name="hp", bufs=1))
        abcp = ctx.enter_context(tc.tile_pool(name="abcp", bufs=1))
        fin = ctx.enter_context(tc.tile_pool(name="fin", bufs=1))
        opool = ctx.enter_context(tc.tile_pool(name="opool", bufs=2))

        vtt = nc.vector.tensor_tensor

        for t in range(N_PASSES):
            X = xpool.tile([128, SLABE], BF16, name="X")
            nc.sync.dma_start(
                X[:, :],
                _dram(xin, t * 128 * SLABE, [[SLABE, 128], [1, SLABE]]),
            )

            # LMH = [lo | mid | hi | u] regions of pitch L; P2 = [pmin|pmax]
            LMH = lmhp.tile([128, 4 * L], BF16, name="LMH")
            P2 = vert.tile([128, 8 * RW], BF16, name="P2")
            lo_o, mid_o, hi_o, u_o = 0, L, 2 * L, 3 * L
            if t == 0:
                for off in (lo_o, mid_o, hi_o):
                    nc.vector.memset(LMH[:, off + OUTE : off + OUTE + 2], 0.0)

            # ---- vertical: column sort3 -> lo, mid, hi ----
            # pairs at slab rows (1,2),(3,4),(5,6),(7,8); output row
            # r = 2k+pol uses pair k and third slab row 2k+3*pol.
            def vert_ops(k0, nk):
                # pair ops for k = k0..k0+nk-1
                pdk = [[RW, nk], [1, RW]]
                xvk = lambda r0: _ap(X, r0 * RW, [[2 * RW, nk], [1, RW]])
                vtt(_ap(P2, k0 * RW, pdk), xvk(2 * k0 + 1), xvk(2 * k0 + 2), MIN)
                vtt(_ap(P2, (4 + k0) * RW, pdk), xvk(2 * k0 + 1), xvk(2 * k0 + 2), MAX)
                vo = [[2 * RW, nk], [RW, 2], [1, RW]]
                vb = [[RW, nk], [0, 2], [1, RW]]
                v3 = [[2 * RW, nk], [3 * RW, 2], [1, RW]]
                pm = _ap(P2, k0 * RW, vb)
                pM = _ap(P2, (4 + k0) * RW, vb)
                x3 = _ap(X, 2 * k0 * RW, v3)
                base = 2 * k0 * RW
                if USE_4D:
                    # {lo,u}: src1 phases {pmin,pmax} (stride 4RW), same x3
                    vtt(
                        _ap(LMH, base, [[3 * L, 2]] + vo),
                        _ap(P2, k0 * RW, [[4 * RW, 2]] + vb),
                        _ap(X, 2 * k0 * RW, [[0, 2]] + v3),
                        MIN,
                    )
                else:
                    vtt(_ap(LMH, lo_o + base, vo), pm, x3, MIN)
                    vtt(_ap(LMH, u_o + base, vo), pM, x3, MIN)
                vtt(_ap(LMH, hi_o + base, vo), pM, x3, MAX)
                vtt(_ap(LMH, mid_o + base, vo), pm, _ap(LMH, u_o + base, vo), MAX)

            vert_ops(0, 4)

            # ---- horizontal pair ops, phase-fused across {lo,mid} and
            # {hi,mid}: H4 = [s_lo | pmx | s_hi | pmn], region pitch R ----
            H4 = hp.tile([128, 4 * R], BF16, name="H4")
            d2 = [[R, 2], [EW, ROWS], [1, EW]]
            s2 = lambda o0, st: _ap(LMH, o0, [[st, 2], [RW, ROWS], [1, EW]])
            vtt(_ap(H4, 0, d2), s2(lo_o + EW, L), s2(lo_o + 1, L), MAX)
            vtt(_ap(H4, 2 * R, d2), s2(hi_o + EW, -L), s2(hi_o + 1, -L), MIN)

            # ---- fused even/odd combines ----
            ABCU = abcp.tile([128, 4 * OUTE], BF16, name="ABCU")
            A_o, B_o, C_o, u2_o = 0, OUTE, 2 * OUTE, 3 * OUTE
            dv2 = [[RW, ROWS], [EW, 2], [1, EW]]
            sbc = lambda o: _ap(H4, o, [[EW, ROWS], [0, 2], [1, EW]])
            tph = lambda o: _ap(LMH, o, [[RW, ROWS], [EW + 1, 2], [1, EW]])
            vtt(_ap(ABCU, A_o, dv2), sbc(0), tph(lo_o), MAX)
            if USE_4D:
                # {Ct, u2}: src1 {s_hi, pmx} (stride -R), src2 {hi, mid} (-L)
                vtt(
                    _ap(ABCU, C_o, [[OUTE, 2]] + dv2),
                    _ap(H4, 2 * R, [[-R, 2], [EW, ROWS], [0, 2], [1, EW]]),
                    _ap(LMH, hi_o, [[-L, 2], [RW, ROWS], [EW + 1, 2], [1, EW]]),
                    MIN,
                )
            else:
                vtt(_ap(ABCU, C_o, dv2), sbc(2 * R), tph(hi_o), MIN)
                vtt(_ap(ABCU, u2_o, dv2), sbc(R), tph(mid_o), MIN)
            vtt(_ap(ABCU, B_o, dv2), sbc(3 * R), _ap(ABCU, u2_o, dv2), MAX)

            # ---- final med3(A, B, C) ----
            flat = [[1, OUTE]]
            mn = fin.tile([128, OUTE], BF16, name="mn")
            mx = fin.tile([128, OUTE], BF16, name="mx")
            t2 = fin.tile([128, OUTE], BF16, name="t2")
            res = opool.tile([128, OUTE], BF16, name="res")
            Av = _ap(ABCU, A_o, flat)
            Bv = _ap(ABCU, B_o, flat)
            vtt(_ap(mn, 0, flat), Av, Bv, MIN)
            vtt(_ap(mx, 0, flat), Av, Bv, MAX)

            vtt(_ap(t2, 0, flat), _ap(mx, 0, flat), _ap(ABCU, C_o, flat), MIN)
            vtt(_ap(res, 0, flat), _ap(mn, 0, flat), _ap(t2, 0, flat), MAX)
            nc.scalar.dma_start(
                _dram(yout, t * 128 * OUTE, [[OUTE, 128], [1, OUTE]]),
                res[:, :],
            )


def _get_nc():
    if "nc" not in _CACHED:
        _install_neff_cache()
        _CACHED["nc"] = _build()
    return _CACHED["nc"]


# staged-input row gather: for each chunk c (0..63), padded rows 8c..8c+9
_ROWIDX = (np.arange(64) * ROWS)[:, None] + np.arange(SLAB)[None, :]


def _stage_input(shard6: np.ndarray) -> np.ndarray:
    """(6, 512, 512) f32 -> [3, 128, SLABE] bf16 E|O staged slabs."""
    eo = np.zeros((PLANES, H + 2, RW), dtype=np.float32)
    # E: padded cols 0,2,...,512 -> [0:257); O: 1,3,...,513 -> [EW:EW+257)
    # padded col j = shard col j-1 for 1<=j<=512
    eo[:, 1:-1, 1:257] = shard6[:, :, 1::2]  # E[k]=col 2k, k=1..256
    eo[:, 1:-1, EW : EW + 256] = shard6[:, :, 0::2]  # O[k]=col 2k+1, k=0..255
    slabs = eo[:, _ROWIDX, :]  # (6, 64, 10, 516)
    return slabs.reshape(N_PASSES, 128, SLABE).astype(ml_dtypes.bfloat16)


def _unstage_output(ys: np.ndarray) -> np.ndarray:
    """[3, 128, OUTE] bf16 -> (6, 512, 512) f32."""
    arr = np.asarray(ys).astype(np.float32)
    arr = arr.reshape(N_PASSES, 2, 64, ROWS, RW).reshape(PLANES, H, RW)
    out = np.empty((PLANES, H, W), dtype=np.float32)
    out[:, :, 0::2] = arr[:, :, 0:256]
    out[:, :, 1::2] = arr[:, :, EW : EW + 256]
    return out


def kernel(image: np.ndarray, _trace: bool = False):
    assert image.shape == (B, C, H, W) and image.dtype == np.float32
    nc = _get_nc()

    per_core = B // N_CORES
    in_maps = []
    for c in range(N_CORES):
        shard = image[c * per_core : (c + 1) * per_core].reshape(PLANES, H, W)
        in_maps.append({"xs": _stage_input(shard)})

    res = run_bass_kernel_spmd(nc, in_maps, list(range(N_CORES)), trace=_trace)
    _CACHED["last_exec_ns"] = res.exec_time_ns

    out = np.empty((B, C, H, W), dtype=np.float32)
    for c in range(N_CORES):
        out[c * per_core : (c + 1) * per_core] = _unstage_output(
            res.results[c]["ys"]
        ).reshape(per_core, C, H, W)
    return out
